# revision 37
# baseline (speedup 1.0000x reference)
"""KDE HyperGraph Conv kernel for 8 Trainium2 NeuronCores.

Math: the reference builds H[i,j] = [rho_i > rho_j] (+ self loop), so every
downstream quantity depends only on the *ranks* of the KDE densities rho.
With r_i = rank(rho_i) (ascending) and g_i = M-1-r_i:

    De_j = g_j + 1,  Dv_i = r_i + 1
    y_i = dvs_i * (  sum_{r_k < r_i} C_k dvs_k x_k
                   + C_i * sum_{r_k >= r_i} dvs_k x_k ),  dvs = Dv^-1/2
    C(g) = H_M - H_g  (harmonic numbers, asymptotic expansion)

which turns the [M,M]@[M,M] propagation into masked [M,M]@[M,C] matmuls with
the 0/1 comparison matrix L[k,j] = [rho_k < rho_j].

Perf structure vs the f32r baseline:
  - L is materialized ONCE per batch as fp8 pair-tiles and consumed by
    fp8 DoubleRow matmuls (2 K-planes per instr at 0.5 cyc/row = 4x) for
    both the rank column-sums and the masked P1/P2 matmuls.  u,v are fed
    hi/lo-split (u8 + r8) in fp8 with static scales, keeping rel err ~2e-3.
  - Partition broadcasts (rho_bc, dvs_bc, msq, ...) moved from K=1 PE
    matmuls + ACT copies to the idle GPSIMD engine's partition_broadcast.
  - sq comes from a DVE square+strided-reduce on xn (no DMA roundtrip).
  - batch 0's rank pipeline (rank matmuls + roundtrip) is interleaved into
    batch 1's distance pass so the PE never idles waiting on the serial
    rank -> scalars -> split chain.
  - final SiLU fused into one ACT op; W@z consumes z as two addends so the
    PSUM P1/P2 tiles free early.

Data-parallel over batch: each of the 8 cores handles B/8 = 2 batches.
"""

import os
import sys

for _p in ("/opt/trn_rl_repo",):
    if os.path.isdir(_p) and _p not in sys.path:
        sys.path.append(_p)

import numpy as np

import concourse.bass as bass
import concourse.tile as tile
from concourse import bacc, mybir
from concourse.bass_utils import run_bass_kernel_spmd
from concourse.masks import make_identity

N_CORES = 8
B, M, C = 16, 2048, 128
NB = B // N_CORES          # batches per core
NT = M // 128              # 128-row chunks per batch
NS = M // 512              # 512-wide column slices
NP = NT // 2               # fp8 pair-tiles per batch
EPS = 0.1                  # diagonal-safety shift inside sqrt
GAMMA = 0.5772156649015329
H_M = float((1.0 / np.arange(1, M + 1, dtype=np.float64)).sum())
SU = 32.0                  # fp8 scale for u = dvs*x
SV = 128.0                 # fp8 scale for v = dvs*C*x
H_TILE = 5                 # sampled tile for the bandwidth estimate
N_GPS_CMP = 8              # fp8 casts offloaded to gpsimd per batch

F32 = mybir.dt.float32
BF16 = mybir.dt.bfloat16
F32R = mybir.dt.float32r
F8 = mybir.dt.float8e4
AF = mybir.ActivationFunctionType
ALU = mybir.AluOpType
AX = mybir.AxisListType
PM = mybir.MatmulPerfMode


def _tb(t):
    return slice(t * 128, (t + 1) * 128)


def _sl(j):
    return slice(j * 512, (j + 1) * 512)


def build_kernel():
    nc = bacc.Bacc("TRN2", target_bir_lowering=False, debug=False)

    # Per-core inputs (host pre-permuted, see make_in_maps below):
    #   xT[b, c, i]          = x[b, i, c]            (channels on partitions)
    #   xN[b, p, t*128 + c]  = x[b, t*128 + p, c]    (rows on partitions)
    #   WT[c, o]             = W[o, c]
    # Output yH[b, o, i] = y[b, i, o]  (host un-permutes)
    xT = nc.declare_dram_parameter("xT", [NB, 128, M], F32R, isOutput=False)
    xN = nc.declare_dram_parameter("xN", [NB, 128, M], F32, isOutput=False)
    WT = nc.declare_dram_parameter("WT", [128, 128], F32R, isOutput=False)
    yH = nc.declare_dram_parameter("yH", [NB, 128, M], F32, isOutput=True)

    with tile.TileContext(nc) as tc:
        from contextlib import ExitStack

        with ExitStack() as ctx:
            cp = ctx.enter_context(tc.tile_pool(name="consts", bufs=1))
            pb = ctx.enter_context(tc.tile_pool(name="big", bufs=2))
            psm = ctx.enter_context(tc.tile_pool(name="small", bufs=2))
            pp = ctx.enter_context(
                tc.tile_pool(name="psum", bufs=2, space=bass.MemorySpace.PSUM)
            )

            # ---------------- constants ----------------
            ident = cp.tile([128, 128], F32, tag="ident")
            make_identity(nc, ident[:, :])
            o128_f = cp.tile([128, 128], F32, tag="o128_f")
            nc.gpsimd.memset(o128_f[:, :], 1.0 / 128.0)
            oneon128_r = cp.tile([128, 128], F32R, tag="oneon128_r")
            nc.vector.tensor_copy(oneon128_r[:, :], o128_f[:, :])
            ones_col_f = cp.tile([128, 2], F32, tag="ones_col_f")
            nc.gpsimd.memset(ones_col_f[:, :], 1.0)
            ones_col_r = cp.tile([128, 2], F32R, tag="ones_col_r")
            nc.vector.tensor_copy(ones_col_r[:, :], ones_col_f[:, :])
            ones_row_f = cp.tile([1, 128], F32, tag="ones_row_f")
            nc.gpsimd.memset(ones_row_f[:, :], 1.0)
            ones_row_r = cp.tile([1, 128], F32R, tag="ones_row_r")
            nc.vector.tensor_copy(ones_row_r[:, :], ones_row_f[:, :])
            ones8 = cp.tile([128, 256], F8, tag="ones8")
            wt = cp.tile([128, 128], F32R, tag="wt")
            nc.sync.dma_start(wt[:, :], WT[:, :])
            wtn = cp.tile([128, 128], F32R, tag="wtn")
            nc.vector.tensor_scalar_mul(wtn[:, :], wt[:, :], -1.0)

            # dense burst of real-shaped matmuls to ramp the PE p-state
            junk = cp.tile([128, 512], BF16, tag="junk")
            nc.gpsimd.memset(junk[:, :], 1.0)
            nc.vector.tensor_copy(ones8[:, :], junk[:, 0:256])
            nc.gpsimd.memset(junk[:, :], 0.5)
            warm_ps = pp.tile([128, 512], F32, tag="big", name="warmps")
            for _w in range(40):
                nc.tensor.matmul(
                    warm_ps[:, :], lhsT=junk[:, 0:128], rhs=junk[:, :],
                    start=True, stop=True, skip_group_check=True,
                )

            st = [dict() for _ in range(NB)]

            # ---------------- phases ----------------
            def prep(b):
                s = st[b]
                xn = pb.tile([128, M], F32, tag="xn", name=f"xn{b}")
                nc.sync.dma_start(xn[:, :], xN[b])
                xt = pb.tile([128, M], F32R, tag="xt", name=f"xt{b}")
                nc.sync.dma_start(xt[:, :], xT[b])
                xnsq = pb.tile([128, M], F32, tag="scratch", bufs=3,
                               name=f"xnsq{b}")
                nc.vector.tensor_mul(xnsq[:, :], xn[:, :], xn[:, :])
                sqc = psm.tile([128, NT], F32, tag="sqc", name=f"sqc{b}")
                nc.vector.tensor_reduce(
                    sqc[:, :], xnsq[:, :].rearrange("p (t c) -> p t c", t=NT),
                    axis=AX.X, op=ALU.add,
                )
                bias_s = psm.tile([128, NT], F32, tag="bias_s", name=f"biass{b}")
                nc.vector.tensor_scalar_add(bias_s[:, :], sqc[:, :], EPS)
                # msq = broadcast of -sq_j/2 (f32r chain: DVE -> PE T -> ACT ->
                # DMA row -> gpsimd partition_broadcast)
                sqm = psm.tile([128, NT], F32, tag="sqm", name=f"sqm{b}")
                nc.vector.tensor_scalar_mul(sqm[:, :], sqc[:, :], -0.5)
                sqmT_ps = pp.tile([NT, 128], F32, tag="big", name=f"sqmT{b}")
                nc.tensor.transpose(sqmT_ps[:, :], sqm[:, :], ident[:, :])
                sqmT = psm.tile([NT, 128], F32R, tag="sqmT", name=f"sqmTs{b}")
                nc.scalar.copy(sqmT[:, :], sqmT_ps[:, :])
                msq_row = psm.tile([1, M], F32R, tag="rowtmp", bufs=2,
                                   name=f"msqrow{b}")
                nc.sync.dma_start(msq_row[0:1, :], sqmT[:, :])
                msq = pb.tile([128, M], F32R, tag="msq", bufs=2, name=f"msq{b}")
                nc.gpsimd.partition_broadcast(msq[:, :], msq_row[0:1, :])
                s["xt"], s["xn"], s["sqc"], s["bias_s"], s["msq"] = (
                    xt, xn, sqc, bias_s, msq,
                )

            def emit_d2(s, t, d2_ps):
                # mains first: they only need xt, while the seed waits on the
                # msq broadcast roundtrip
                for j in range(NS):
                    nc.tensor.matmul(
                        d2_ps[:, _sl(j)], lhsT=s["xt"][:, _tb(t)],
                        rhs=s["xt"][:, _sl(j)], start=True, stop=False,
                    )
                for j in range(NS):
                    nc.tensor.matmul(
                        d2_ps[:, _sl(j)], lhsT=oneon128_r[:, :],
                        rhs=s["msq"][:, _sl(j)], start=False, stop=True,
                    )

            def passB(b):
                s = st[b]
                hacc = psm.tile([128, 1], F32, tag="hacc", name=f"hacc{b}")
                d2_ps = pp.tile([128, M], F32, tag="big", name=f"d2b{b}")
                emit_d2(s, H_TILE, d2_ps)
                scr = pb.tile([128, M], F8, tag="scr", bufs=1, name=f"sb{b}")
                nc.scalar.activation(
                    scr[:, :], d2_ps[:, :], AF.Sqrt, scale=-2.0,
                    bias=s["bias_s"][:, H_TILE : H_TILE + 1],
                    accum_out=hacc[:, 0:1],
                )
                s["hacc"] = hacc

            def hchain(b):
                s = st[b]
                tot_ps = pp.tile([1, 1], F32, tag="big", name=f"tot{b}")
                nc.tensor.matmul(
                    tot_ps[:, :], lhsT=ones_col_f[:, 0:1], rhs=s["hacc"][:, :]
                )
                ht = psm.tile([1, 1], F32, tag="ht", name=f"ht{b}")
                nc.vector.tensor_scalar(
                    ht[:, :], tot_ps[:, :], 1.0 / (128 * (M - 1)), 1e-6,
                    op0=ALU.mult, op1=ALU.max,
                )
                h2 = psm.tile([1, 1], F32, tag="h2", name=f"h2{b}")
                nc.vector.tensor_mul(h2[:, :], ht[:, :], ht[:, :])
                rh2 = psm.tile([1, 1], F32, tag="rh2", name=f"rh2{b}")
                nc.vector.reciprocal(rh2[:, :], h2[:, :])
                f_bc = psm.tile([128, 1], F32, tag="fbc", name=f"fbc{b}")
                nc.gpsimd.partition_broadcast(f_bc[:, :], rh2[0:1, :])
                bias_e = psm.tile([128, NT], F32, tag="bias_e", name=f"biase{b}")
                nc.vector.tensor_scalar(
                    bias_e[:, :], s["sqc"][:, :], f_bc[:, 0:1], -0.5,
                    op0=ALU.mult, op1=ALU.mult,
                )
                s["f_bc"], s["bias_e"] = f_bc, bias_e

            def passC(b, t0, t1):
                s = st[b]
                if "rho" not in s:
                    s["rho"] = psm.tile([128, NT], F32, tag="rho",
                                        name=f"rho{b}")
                for t in range(t0, t1):
                    d2_ps = pp.tile([128, M], F32, tag="big", name=f"d2c{b}_{t}")
                    emit_d2(s, t, d2_ps)
                    scr = pb.tile([128, M], F8, tag="scr", bufs=1,
                                  name=f"sc{b}_{t}")
                    nc.scalar.activation(
                        scr[:, :], d2_ps[:, :], AF.Exp,
                        scale=s["f_bc"][:, :],
                        bias=s["bias_e"][:, t : t + 1],
                        accum_out=s["rho"][:, t : t + 1],
                    )

            def layout_half(b, h):
                """rho[:, 8h:8h+8] -> row -> rho_bc[:, 1024h:1024h+1024]."""
                s = st[b]
                if "rho_bc" not in s:
                    s["rho_bc"] = pb.tile([128, M], F32, tag="rho_bc", bufs=1,
                                          name=f"rbc{b}")
                hs = slice(8 * h, 8 * h + 8)
                cs = slice(1024 * h, 1024 * h + 1024)
                rT_ps = pp.tile([8, 128], F32, tag="big", name=f"rTps{b}_{h}")
                nc.tensor.transpose(rT_ps[:, :], s["rho"][:, hs], ident[:, :])
                rT = psm.tile([8, 128], F32, tag="rT", name=f"rT{b}_{h}")
                nc.vector.tensor_copy(rT[:, :], rT_ps[:, :])
                rrow = psm.tile([1, 1024], F32, tag="rowh", name=f"rrow{b}_{h}")
                nc.sync.dma_start(rrow[0:1, :], rT[:, :])
                nc.gpsimd.partition_broadcast(s["rho_bc"][:, cs], rrow[0:1, :])

            def layout(b):
                layout_half(b, 0)
                layout_half(b, 1)

            def compares(b):
                """lt pair-tiles, fp8: ltp[s][p, i, j] = [rho_j > rho_(2s+i)*128+p].
                DVE is_gt with fp8 output is ~9x slow, so compare to f32r then
                cast via tensor_copy (fast).  Big gpsimd ops starve the DVE of
                SBUF ports, so casts go to DVE (batch 0, hidden under the other
                batch's distance pass) or alternate DVE/ACT (batch 1 tail,
                where ACT is idle)."""
                s = st[b]
                if "ltp" not in s:
                    s["ltp"] = [
                        pb.tile([128, 2, M], F8, tag="ltp", bufs=2 * NP,
                                name=f"ltp{b}_{p8}")
                        for p8 in range(NP)
                    ]
                done = s.setdefault("cmp_done", set())
                for p8 in range(NP):
                    lt = s["ltp"][p8]
                    for i, t in ((0, 2 * p8), (1, 2 * p8 + 1)):
                        cs = slice(0, M)
                        if (t, 0) in done:
                            cs = slice(1024, M)
                        elif (t, 1) in done:
                            cs = slice(0, 1024)
                        w = cs.stop - cs.start
                        lt32 = pb.tile([128, M], F32R, tag="scratch", bufs=3,
                                       name=f"lt32_{b}_{t}_{cs.start}")
                        nc.vector.tensor_scalar(
                            lt32[:, 0:w], s["rho_bc"][:, cs],
                            s["rho"][:, t : t + 1], None, op0=ALU.is_gt,
                        )
                        if b == 1 and t % 3 != 0:
                            nc.scalar.copy(lt[:, i, cs], lt32[:, 0:w])
                        else:
                            nc.vector.tensor_copy(lt[:, i, cs], lt32[:, 0:w])
                        done.add((t, 0))
                        if cs.stop == M:
                            done.add((t, 1))

            def compares_front(b):
                """left-half compares for tiles 0..7, run while the other
                half of rho is still being produced (DVE idle window)."""
                s = st[b]
                if "ltp" not in s:
                    s["ltp"] = [
                        pb.tile([128, 2, M], F8, tag="ltp", bufs=2 * NP,
                                name=f"ltp{b}_{p8}")
                        for p8 in range(NP)
                    ]
                done = s.setdefault("cmp_done", set())
                for p8 in range(4):
                    lt = s["ltp"][p8]
                    for i, t in ((0, 2 * p8), (1, 2 * p8 + 1)):
                        lt32 = pb.tile([128, M], F32R, tag="scratch", bufs=3,
                                       name=f"lt32f_{b}_{t}")
                        nc.vector.tensor_scalar(
                            lt32[:, 0:1024], s["rho_bc"][:, 0:1024],
                            s["rho"][:, t : t + 1], None, op0=ALU.is_gt,
                        )
                        nc.vector.tensor_copy(lt[:, i, 0:1024], lt32[:, 0:1024])
                        done.add((t, 0))

            def gstream_mm(b):
                s = st[b]
                rank_ps = pp.tile([128, M], F32, tag="big", name=f"rankps{b}")
                for p8 in range(NP):
                    for j in range(NS):
                        nc.tensor.matmul(
                            rank_ps[:, _sl(j)],
                            lhsT=ones8[:, :].rearrange("p (two c) -> p two c",
                                                       two=2),
                            rhs=s["ltp"][p8][:, :, _sl(j)],
                            start=(p8 == 0), stop=(p8 == NP - 1),
                            perf_mode=PM.DoubleRow,
                        )
                s["rank_ps"] = rank_ps
                r_sb = psm.tile([1, M], F32, tag="rowtmp", bufs=2,
                                name=f"rsb{b}")
                if b == 1:
                    nc.vector.tensor_copy(r_sb[0:1, :], rank_ps[0:1, :])
                else:
                    nc.scalar.copy(r_sb[0:1, :], rank_ps[0:1, :])
                r16 = psm.tile([NT, 128], F32, tag="r16", name=f"r16_{b}")
                nc.sync.dma_start(r16[:, :], r_sb[0:1, :])
                s["r16"] = r16

            def gstream_fin(b):
                s = st[b]
                rc_ps = pp.tile([128, NT], F32, tag="big", name=f"rcps{b}")
                nc.tensor.transpose(rc_ps[:, :], s["r16"][:, :],
                                    ident[0:NT, 0:NT])
                racc = psm.tile([128, NT], F32, tag="racc", name=f"racc{b}")
                nc.scalar.copy(racc[:, :], rc_ps[:, :])
                s["racc"] = racc

            def scalars(b):
                s = st[b]
                racc = s["racc"]
                Dv = psm.tile([128, NT], F32, tag="Dv", name=f"Dv{b}")
                nc.vector.tensor_scalar_add(Dv[:, :], racc[:, :], 1.0)
                g = psm.tile([128, NT], F32, tag="g", name=f"g{b}")
                nc.vector.tensor_scalar(
                    g[:, :], racc[:, :], -1.0, float(M - 1), op0=ALU.mult,
                    op1=ALU.add,
                )
                gm = psm.tile([128, NT], F32, tag="gm", name=f"gm{b}")
                nc.vector.tensor_scalar_max(gm[:, :], g[:, :], 1.0)
                lnDv = psm.tile([128, NT], F32, tag="lnDv", name=f"lnDv{b}")
                nc.scalar.activation(lnDv[:, :], Dv[:, :], AF.Ln)
                lng = psm.tile([128, NT], F32, tag="lng", name=f"lng{b}")
                nc.scalar.activation(lng[:, :], gm[:, :], AF.Ln)
                dvs = psm.tile([128, NT], F32, tag="dvs", name=f"dvs{b}")
                nc.scalar.activation(dvs[:, :], lnDv[:, :], AF.Exp, scale=-0.5)
                inv = psm.tile([128, NT], F32, tag="inv", name=f"inv{b}")
                nc.vector.reciprocal(inv[:, :], gm[:, :])
                inv2 = psm.tile([128, NT], F32, tag="inv2", name=f"inv2{b}")
                nc.vector.tensor_mul(inv2[:, :], inv[:, :], inv[:, :])
                c1 = psm.tile([128, NT], F32, tag="c1", name=f"c1{b}")
                nc.vector.tensor_scalar(
                    c1[:, :], lng[:, :], -1.0, H_M - GAMMA, op0=ALU.mult,
                    op1=ALU.add,
                )
                c2 = psm.tile([128, NT], F32, tag="c2", name=f"c2{b}")
                nc.vector.scalar_tensor_tensor(
                    c2[:, :], in0=inv[:, :], scalar=-0.5, in1=c1[:, :],
                    op0=ALU.mult, op1=ALU.add,
                )
                Cv = psm.tile([128, NT], F32, tag="Cv", name=f"Cv{b}")
                nc.vector.scalar_tensor_tensor(
                    Cv[:, :], in0=inv2[:, :], scalar=1.0 / 12.0, in1=c2[:, :],
                    op0=ALU.mult, op1=ALU.add,
                )
                dvsC = psm.tile([128, NT], F32, tag="dvsC", name=f"dvsC{b}")
                nc.vector.tensor_mul(dvsC[:, :], dvs[:, :], Cv[:, :])

                # scaled broadcast rows: dvs/SV and dvsC/SU
                stk = psm.tile([128, 2 * NT], F32, tag="stk", name=f"stk{b}")
                nc.vector.tensor_scalar_mul(stk[:, 0:NT], dvs[:, :], 1.0 / SV)
                nc.vector.tensor_scalar_mul(stk[:, NT : 2 * NT], dvsC[:, :],
                                            1.0 / SU)
                stT_ps = pp.tile([2 * NT, 128], F32, tag="big", name=f"stT{b}")
                nc.tensor.transpose(stT_ps[:, :], stk[:, :], ident[:, :])
                stT = psm.tile([2 * NT, 128], F32R, tag="stTs", name=f"stTs{b}")
                nc.scalar.copy(stT[:, :], stT_ps[:, :])
                dvs_row = psm.tile([1, M], F32R, tag="rowtmp", bufs=2,
                                   name=f"dr{b}")
                nc.sync.dma_start(dvs_row[0:1, :], stT[0:NT, :])
                dvsC_row = psm.tile([1, M], F32R, tag="rowtmp", bufs=2,
                                    name=f"cr{b}")
                nc.sync.dma_start(dvsC_row[0:1, :], stT[NT : 2 * NT, :])
                dbc_ps = pp.tile([128, M], F32, tag="big", name=f"dbcps{b}")
                for j in range(NS):
                    nc.tensor.matmul(
                        dbc_ps[:, _sl(j)], lhsT=ones_row_r[0:1, :],
                        rhs=dvs_row[0:1, _sl(j)],
                    )
                dvs_bc = pb.tile([128, M], F32, tag="dvs_bc", bufs=1,
                                 name=f"db{b}")
                nc.scalar.copy(dvs_bc[:, :], dbc_ps[:, :])
                cbc_ps = pp.tile([128, M], F32, tag="big", name=f"cbcps{b}")
                for j in range(NS):
                    nc.tensor.matmul(
                        cbc_ps[:, _sl(j)], lhsT=ones_row_r[0:1, :],
                        rhs=dvsC_row[0:1, _sl(j)],
                    )
                dvsC_bc = pb.tile([128, M], F32, tag="dvsC_bc", bufs=1,
                                  name=f"cb{b}")
                nc.scalar.copy(dvsC_bc[:, :], cbc_ps[:, :])
                s["dvs_bc"], s["dvsC_bc"] = dvs_bc, dvsC_bc

                # u32 = SU*dvs*x, v128 = SV*dvsC*x  (per-tile broadcast APs)
                dvsU = psm.tile([128, NT], F32, tag="dvsU", name=f"dvsU{b}")
                nc.vector.tensor_scalar_mul(dvsU[:, :], dvs[:, :], SU)
                dvsCV = psm.tile([128, NT], F32, tag="dvsCV", name=f"dvsCV{b}")
                nc.vector.tensor_scalar_mul(dvsCV[:, :], dvsC[:, :], SV)
                u32 = pb.tile([128, M], F32, tag="scratch", bufs=3,
                              name=f"u32_{b}")
                nc.vector.tensor_tensor(
                    u32[:, :].rearrange("p (t c) -> p t c", t=NT),
                    s["xn"][:, :].rearrange("p (t c) -> p t c", t=NT),
                    dvsU[:, :].unsqueeze(2).broadcast_to([128, NT, 128]),
                    op=ALU.mult,
                )
                v128 = pb.tile([128, M], F32, tag="scratch", bufs=3,
                               name=f"v128_{b}")
                nc.vector.tensor_tensor(
                    v128[:, :].rearrange("p (t c) -> p t c", t=NT),
                    s["xn"][:, :].rearrange("p (t c) -> p t c", t=NT),
                    dvsCV[:, :].unsqueeze(2).broadcast_to([128, NT, 128]),
                    op=ALU.mult,
                )
                # fp8 hi/lo splits, per pair-tile so l2 starts on pair 0
                # (residual via f32 sub + fast cast; fp8-out tensor_tensor is
                # ~2.6x slower on DVE)
                u8 = pb.tile([128, M], F8, tag="fp8s", bufs=4, name=f"u8_{b}")
                r8 = pb.tile([128, M], F8, tag="fp8s", bufs=4, name=f"r8_{b}")
                v8 = pb.tile([128, M], F8, tag="fp8s", bufs=4, name=f"v8_{b}")
                s8 = pb.tile([128, M], F8, tag="fp8s", bufs=4, name=f"s8_{b}")
                r32 = pb.tile([128, M], F32, tag="scratch", bufs=3,
                              name=f"r32_{b}")
                s32 = pb.tile([128, M], F32, tag="scratch", bufs=3,
                              name=f"s32_{b}")
                for p8 in range(NP):
                    ps = slice(2 * p8 * 128, (2 * p8 + 2) * 128)
                    nc.vector.tensor_copy(u8[:, ps], u32[:, ps])
                    nc.vector.tensor_sub(r32[:, ps], u32[:, ps], u8[:, ps])
                    nc.vector.tensor_copy(r8[:, ps], r32[:, ps])
                    nc.vector.tensor_copy(v8[:, ps], v128[:, ps])
                    nc.vector.tensor_sub(s32[:, ps], v128[:, ps], v8[:, ps])
                    nc.vector.tensor_copy(s8[:, ps], s32[:, ps])
                s["u8"], s["r8"], s["v8"], s["s8"] = u8, r8, v8, s8

                # T32 = SU * colsum(u) : strided reduce + one matmul
                ured = psm.tile([128, 128], F32R, tag="ured", name=f"ured{b}")
                with nc.allow_low_precision(reason="f32r == f32 bits"):
                    nc.vector.tensor_reduce(
                        ured[:, :],
                        u32[:, :].rearrange("p (t c) -> p c t", t=NT),
                        axis=AX.X, op=ALU.add,
                    )
                T_ps = pp.tile([128, 2], F32, tag="big", name=f"Tps{b}")
                nc.tensor.matmul(T_ps[:, :], lhsT=ured[:, :],
                                 rhs=ones_col_r[:, :])
                T_sb = psm.tile([128, 1], F32, tag="T_sb", name=f"Tsb{b}")
                nc.vector.tensor_copy(T_sb[:, :], T_ps[:, 0:1])
                s["T_sb"] = T_sb

            def pair_lhs(t8, p8):
                return t8[:, 2 * p8 * 128 : (2 * p8 + 2) * 128].rearrange(
                    "p (two c) -> p two c", two=2
                )

            def l2(b):
                """P2' = L@(u8+r8), P1' = L@(v8+s8) via fp8 DoubleRow; the
                zt addends are built as soon as each PSUM finishes."""
                s = st[b]
                P2_ps = pp.tile([128, M], F32, tag="big", name=f"P2ps{b}")
                for hi, t8 in ((0, s["u8"]), (1, s["r8"])):
                    for p8 in range(NP):
                        for j in range(NS):
                            nc.tensor.matmul(
                                P2_ps[:, _sl(j)], lhsT=pair_lhs(t8, p8),
                                rhs=s["ltp"][p8][:, :, _sl(j)],
                                start=(hi == 0 and p8 == 0),
                                stop=(hi == 1 and p8 == NP - 1),
                                perf_mode=PM.DoubleRow,
                            )
                # zt1 = (P2' - T32) * dvsC/SU   (frees P2 early)
                zt1 = pb.tile([128, M], F32R, tag="scratch", bufs=3,
                              name=f"zt1{b}")
                nc.vector.scalar_tensor_tensor(
                    zt1[:, :], in0=P2_ps[:, :], scalar=s["T_sb"][:, 0:1],
                    in1=s["dvsC_bc"][:, :], op0=ALU.subtract, op1=ALU.mult,
                )
                P1_ps = pp.tile([128, M], F32, tag="big", name=f"P1ps{b}")
                for hi, t8 in ((0, s["v8"]), (1, s["s8"])):
                    for p8 in range(NP):
                        for j in range(NS):
                            nc.tensor.matmul(
                                P1_ps[:, _sl(j)], lhsT=pair_lhs(t8, p8),
                                rhs=s["ltp"][p8][:, :, _sl(j)],
                                start=(hi == 0 and p8 == 0),
                                stop=(hi == 1 and p8 == NP - 1),
                                perf_mode=PM.DoubleRow,
                            )
                zt2 = pb.tile([128, M], F32R, tag="scratch", bufs=3,
                              name=f"zt2{b}")
                nc.vector.tensor_mul(zt2[:, :], P1_ps[:, :], s["dvs_bc"][:, :])
                s["zt1"], s["zt2"] = zt1, zt2

            def proj(b):
                """yT = W@zt2 - W@zt1 accumulated in PSUM; SiLU; store."""
                s = st[b]
                yT_ps = pp.tile([128, M], F32, tag="big", name=f"yTps{b}")
                for j in range(NS):
                    nc.tensor.matmul(
                        yT_ps[:, _sl(j)], lhsT=wt[:, :], rhs=s["zt2"][:, _sl(j)],
                        start=True, stop=False,
                    )
                for j in range(NS):
                    nc.tensor.matmul(
                        yT_ps[:, _sl(j)], lhsT=wtn[:, :], rhs=s["zt1"][:, _sl(j)],
                        start=False, stop=True,
                    )
                y_sb = pb.tile([128, M], F32, tag="scratch", bufs=3,
                               name=f"ysb{b}")
                for h in range(2):
                    hs = slice(1024 * h, 1024 * h + 1024)
                    nc.scalar.activation(y_sb[:, hs], yT_ps[:, hs], AF.Silu)
                    nc.sync.dma_start(yH[b][:, hs], y_sb[:, hs])

            # ---------------- schedule ----------------
            prep(0)
            prep(1)
            passB(0)
            passB(1)
            hchain(0)
            passC(0, 0, 1)
            hchain(1)
            passC(0, 1, 9)
            layout_half(0, 0)
            passC(0, 9, NT)
            compares_front(0)
            layout_half(0, 1)
            compares(0)
            passC(1, 0, 8)
            gstream_mm(0)          # rank DRs + row copy interleave into C1
            passC(1, 8, 12)
            gstream_fin(0)
            passC(1, 12, NT)
            layout(1)
            scalars(0)
            compares(1)
            l2(0)
            gstream_mm(1)
            gstream_fin(1)
            proj(0)
            scalars(1)
            l2(1)
            proj(1)

    nc.compile()
    return nc


_CACHED_NC = None


def _get_nc():
    global _CACHED_NC
    if _CACHED_NC is None:
        _CACHED_NC = build_kernel()
    return _CACHED_NC


def make_in_maps(x, W):
    x = np.asarray(x, dtype=np.float32)
    W = np.asarray(W, dtype=np.float32)
    wt = np.ascontiguousarray(W.T)
    in_maps = []
    for core in range(N_CORES):
        xb = x[core * NB : (core + 1) * NB]                       # [NB, M, C]
        xt = np.ascontiguousarray(xb.transpose(0, 2, 1))          # [NB, C, M]
        xn = np.ascontiguousarray(
            xb.reshape(NB, NT, 128, C).transpose(0, 2, 1, 3).reshape(NB, 128, M)
        )
        in_maps.append({"xT": xt, "xN": xn, "WT": wt})
    return in_maps


def unshard_output(results):
    outs = []
    for core in range(N_CORES):
        yh = results[core]["yH"]                                  # [NB, C, M]
        outs.append(yh.transpose(0, 2, 1))                        # [NB, M, C]
    return np.concatenate(outs, axis=0).astype(np.float32)


def run(x, W, trace=False, trace_kwargs=None):
    nc = _get_nc()
    res = run_bass_kernel_spmd(
        nc,
        make_in_maps(x, W),
        list(range(N_CORES)),
        trace=trace,
        **(trace_kwargs or {}),
    )
    return unshard_output(res.results), res


def kernel(x, W):
    y, _ = run(x, W, trace=False)
    return y
